# revision 1
# baseline (speedup 1.0000x reference)
"""GAT (3-layer, 8-head) forward on 8 Trainium2 NeuronCores.

Strategy (graph/data parallel, per sharding hint):
  - Nodes are sharded across 8 cores (2500 -> padded 2560 rows each).
  - Per layer: each core computes h = x @ W for its node shard (feature-major
    GEMM), plus attention logits e_src/e_dst via an embedded (D,16) matrix.
  - AllGather of [h | e_src] (node-major rows) across cores.
  - Edge phase per core: edges are pre-sorted by destination block (host-side,
    index manipulation only).  For each 128-edge chunk: indirect-DMA gather of
    source rows, e = leaky_relu(e_src + e_dst), ee = exp(e), and a 0/1
    mask-matmul on the TensorEngine performs the segment (scatter-add)
    reduction of both ee*h and ee into PSUM.  Softmax normalization happens
    after aggregation (exact up to fp reassociation; max-subtraction is not
    needed since |e| is O(1)).
  - LayerNorm+ReLU epilogue per 128-node block, then an on-chip transpose
    produces the feature-major x for the next layer.
  - Mean-pool by graph via host-built 0/1 pool-mask matmuls + AllReduce, then
    the FC head (replicated) on every core.
"""

import os
import sys

sys.path.insert(0, "/opt/trn_rl_repo")

import numpy as np

import concourse.bass as bass
import concourse.mybir as mybir
import concourse.tile as tile
from concourse import bacc
from concourse.bass_utils import run_bass_kernel_spmd
from concourse.masks import make_identity

F32 = mybir.dt.float32
I32 = mybir.dt.int32
ALU = mybir.AluOpType
ACT = mybir.ActivationFunctionType

P = 128

# Results of the last run (for test harnesses).
LAST_RESULTS = None


def _full_cfg():
    return dict(
        n_cores=8,
        N=20000,
        D=512,
        H=8,
        G=128,
        OUT=128,
        NEG=0.2,
        EPS=1e-5,
        L=3,
    )


# --------------------------------------------------------------------------
# Host-side preprocessing: pure index manipulation / relayout (no float math
# on tensor data beyond dtype casts and zero padding).
# --------------------------------------------------------------------------


def _prep(inputs, cfg):
    nc_ = cfg["n_cores"]
    N, D, H, G, OUT, L = cfg["N"], cfg["D"], cfg["H"], cfg["G"], cfg["OUT"], cfg["L"]
    C = D // H

    x = np.asarray(inputs["x"], np.float32)
    ei = np.asarray(inputs["edge_index"])
    batch = np.asarray(inputs["batch"]).astype(np.int64)

    SHR = (N + nc_ - 1) // nc_  # real nodes per core
    SH = ((SHR + P - 1) // P) * P  # padded nodes per core
    NB = SH // P
    KD = D // P

    # --- edges with self loops
    loops = np.arange(N, dtype=np.int64)
    src = np.concatenate([ei[0], loops])
    dst = np.concatenate([ei[1], loops])
    # padded global id
    pid = (src // SHR) * SH + (src % SHR)
    dcore = dst // SHR
    dloc = dst % SHR

    # --- group edges by (core, dst block)
    per_core_blocks = []  # [core][block] -> (src_pid array, dst_local array)
    maxch = 1
    for c in range(nc_):
        sel = np.nonzero(dcore == c)[0]
        dl = dloc[sel]
        blk = dl // P
        order = np.argsort(blk, kind="stable")
        sel = sel[order]
        dl = dl[order]
        blk = blk[order]
        blocks = []
        for b in range(NB):
            m = blk == b
            s_p = pid[sel[m]]
            d_l = dl[m] % P
            blocks.append((s_p, d_l))
            maxch = max(maxch, (len(s_p) + P - 1) // P)
        per_core_blocks.append(blocks)
    CH = maxch

    gidx = np.zeros((nc_, P, NB, CH), np.int32)
    masks = np.zeros((nc_, NB, P, CH, P), np.float32)
    maskTs = np.zeros((nc_, NB, P, CH, P), np.float32)
    for c in range(nc_):
        for b in range(NB):
            s_p, d_l = per_core_blocks[c][b]
            n = len(s_p)
            j = np.arange(n)
            ch = j // P
            jj = j % P
            gidx[c, jj, b, ch] = s_p
            masks[c, b, jj, ch, d_l] = 1.0
            maskTs[c, b, d_l, ch, jj] = 1.0

    # --- x shards, feature-major (KD, 128, SH)
    xT = np.zeros((nc_, KD, P, SH), np.float32)
    for c in range(nc_):
        rows = x[c * SHR : min((c + 1) * SHR, N)]
        xp = np.zeros((SH, D), np.float32)
        xp[: len(rows)] = rows
        xT[c] = xp.T.reshape(KD, P, SH)

    # --- weights
    W_all = np.zeros((L, KD, P, D), np.float32)
    WT_all = np.zeros((L, KD, P, D), np.float32)
    A_all = np.zeros((L, KD, P, 2 * H), np.float32)
    b_l, g_l, be_l = [], [], []
    for l in range(L):
        W = np.asarray(inputs[f"W{l}"], np.float32)
        W_all[l] = W.reshape(KD, P, D)
        WT_all[l] = np.ascontiguousarray(W.T).reshape(KD, P, D)
        A = np.zeros((D, 2 * H), np.float32)
        a_s = np.asarray(inputs[f"as{l}"], np.float32)
        a_d = np.asarray(inputs[f"ad{l}"], np.float32)
        for h in range(H):
            A[h * C : (h + 1) * C, h] = a_s[h]
            A[h * C : (h + 1) * C, H + h] = a_d[h]
        A_all[l] = A.reshape(KD, P, 2 * H)
        b_l.append(np.asarray(inputs[f"b{l}"], np.float32))
        g_l.append(np.asarray(inputs[f"g{l}"], np.float32))
        be_l.append(np.asarray(inputs[f"be{l}"], np.float32))

    skip_b = all(not b.any() for b in b_l)
    skip_g = all((g == 1.0).all() for g in g_l)
    skip_be = all(not be.any() for be in be_l)

    fc_W = np.asarray(inputs["fc_W"], np.float32).reshape(KD, P, OUT)
    fc_b = np.asarray(inputs["fc_b"], np.float32)
    skip_fcb = not fc_b.any()

    # --- pool masks (0/1 membership)
    poolmask = np.zeros((nc_, NB, P, G), np.float32)
    for c in range(nc_):
        lo = c * SHR
        hi = min((c + 1) * SHR, N)
        loc = np.arange(hi - lo)
        g_of = batch[lo:hi]
        poolmask[c, loc // P, loc % P, g_of] = 1.0

    meta = dict(
        SH=SH, NB=NB, KD=KD, CH=CH, ROW=D + H,
        skip_b=skip_b, skip_g=skip_g, skip_be=skip_be, skip_fcb=skip_fcb,
    )

    in_maps = []
    for c in range(nc_):
        m = dict(
            xT=xT[c],
            W_all=W_all,
            WT_all=WT_all,
            A_all=A_all,
            fc_W=fc_W,
            gidx=gidx[c],
            masks=masks[c],
            maskTs=maskTs[c],
            poolmask=poolmask[c],
        )
        if not skip_b:
            m["b_rep"] = np.broadcast_to(
                np.stack(b_l)[:, None, :], (L, P, D)
            ).copy()
        if not skip_g:
            m["g_rep"] = np.broadcast_to(
                np.stack(g_l)[:, None, :], (L, P, D)
            ).copy()
        if not skip_be:
            m["be_rep"] = np.broadcast_to(
                np.stack(be_l)[:, None, :], (L, P, D)
            ).copy()
        if not skip_fcb:
            m["fcb_rep"] = np.broadcast_to(fc_b[None, :], (P, OUT)).copy()
        in_maps.append(m)
    return in_maps, meta


# --------------------------------------------------------------------------
# Device program
# --------------------------------------------------------------------------


def build(tc, cfg, meta, I, out_ap):
    """I: dict name -> AP (ExternalInputs); out_ap: ExternalOutput (G, OUT)."""
    nc = tc.nc
    nc_cores = cfg["n_cores"]
    D, H, G, OUT, L = cfg["D"], cfg["H"], cfg["G"], cfg["OUT"], cfg["L"]
    NEG, EPS = cfg["NEG"], cfg["EPS"]
    C = D // H
    SH, NB, KD, CH, ROW = meta["SH"], meta["NB"], meta["KD"], meta["CH"], meta["ROW"]
    H2 = 2 * H
    NS = (SH + 511) // 512  # 512-wide node slices for the heT GEMM

    rg = [list(range(nc_cores))]
    shared = "Shared" if nc_cores > 4 else "Local"

    from contextlib import ExitStack

    ctx = ExitStack()
    res = ctx.enter_context(tc.tile_pool(name="res", bufs=1))
    dram = ctx.enter_context(tc.tile_pool(name="dram", bufs=1, space="DRAM"))
    psum = ctx.enter_context(tc.tile_pool(name="psum", bufs=1, space="PSUM"))
    sb = ctx.enter_context(tc.tile_pool(name="sb", bufs=1))

    # ---------------- resident tiles
    xT_sb = [res.tile([P, SH], F32, name=f"xT{k}") for k in range(KD)]
    xn_sb = [res.tile([P, D], F32, name=f"xn{b}") for b in range(NB)]
    henm_sb = [res.tile([P, H2], F32, name=f"henm{b}") for b in range(NB)]
    heT_sb = res.tile([H2, SH], F32, name="heT")
    gidx_sb = res.tile([P, NB, CH], I32, name="gidx")
    W_sb = [res.tile([P, D], F32, name=f"W{k}") for k in range(KD)]
    WT_sb = [res.tile([P, D], F32, name=f"WT{k}") for k in range(KD)]
    A_sb = [res.tile([P, H2], F32, name=f"A{k}") for k in range(KD)]
    wa_sb = [res.tile([P, H2], F32, name=f"wa{k}") for k in range(KD)]
    id128 = res.tile([P, P], F32, name="id128")
    idh2 = res.tile([H2, H2], F32, name="idh2")
    make_identity(nc, id128[:])
    make_identity(nc, idh2[:])
    eps_sb = res.tile([P, 1], F32, name="eps_sb")
    nc.vector.memset(eps_sb[:], float(EPS))
    if G == P:
        idG = id128
    else:
        idG = res.tile([G, G], F32, name="idG")
        make_identity(nc, idG[:])

    b_rep = g_rep = be_rep = fcb_rep = None
    if not meta["skip_b"]:
        b_rep = res.tile([P, D], F32, name="b_rep")
    if not meta["skip_g"]:
        g_rep = res.tile([P, D], F32, name="g_rep")
    if not meta["skip_be"]:
        be_rep = res.tile([P, D], F32, name="be_rep")

    nc.sync.dma_start(out=gidx_sb[:], in_=I["gidx"][:])
    for k in range(KD):
        nc.sync.dma_start(out=xT_sb[k][:], in_=I["xT"][k])

    # ---------------- DRAM comm buffers
    ag_in = dram.tile([SH, ROW], F32, name="ag_in")
    ag_outs = [
        dram.tile([nc_cores * SH, ROW], F32, name=f"ag_out{l}", addr_space=shared)
        for l in range(L)
    ]
    ar_in = dram.tile([G, D + 1], F32, name="ar_in")
    ar_out = dram.tile([G, D + 1], F32, name="ar_out", addr_space=shared)

    for l in range(L):
        ag_out = ag_outs[l]
        # ---------- load layer weights
        for k in range(KD):
            nc.sync.dma_start(out=W_sb[k][:], in_=I["W_all"][l, k])
            nc.sync.dma_start(out=WT_sb[k][:], in_=I["WT_all"][l, k])
            nc.sync.dma_start(out=A_sb[k][:], in_=I["A_all"][l, k])
        if b_rep is not None:
            nc.sync.dma_start(out=b_rep[:], in_=I["b_rep"][l])
        if g_rep is not None:
            nc.sync.dma_start(out=g_rep[:], in_=I["g_rep"][l])
        if be_rep is not None:
            nc.sync.dma_start(out=be_rep[:], in_=I["be_rep"][l])

        # ---------- GEMM-A: h (node-major) per block -> ag_in[:, 0:D]
        for b in range(NB):
            h_ps = psum.tile([P, D], F32, name="big_ps", tag="big", bufs=2)
            for k in range(KD):
                nc.tensor.matmul(
                    out=h_ps[:],
                    lhsT=xT_sb[k][:, b * P : (b + 1) * P],
                    rhs=W_sb[k][:],
                    start=(k == 0),
                    stop=(k == KD - 1),
                )
            h_sb = sb.tile([P, D], F32, name="h_sb", tag="h_sb", bufs=3)
            nc.vector.tensor_copy(out=h_sb[:], in_=h_ps[:])
            nc.sync.dma_start(out=ag_in[b * P : (b + 1) * P, 0:D], in_=h_sb[:])

        # ---------- WA = W @ A   (KD chunks of (128, 2H))
        for ic in range(KD):
            wa_ps = psum.tile([P, H2], F32, name="wa_ps", tag="ed", bufs=2)
            for oc in range(KD):
                nc.tensor.matmul(
                    out=wa_ps[:],
                    lhsT=WT_sb[oc][:, ic * P : (ic + 1) * P],
                    rhs=A_sb[oc][:],
                    start=(oc == 0),
                    stop=(oc == KD - 1),
                )
            nc.vector.tensor_copy(out=wa_sb[ic][:], in_=wa_ps[:])

        # ---------- heT = (x @ WA).T  (2H x SH)
        for s in range(NS):
            n0 = s * 512
            n1 = min(SH, n0 + 512)
            he_ps = psum.tile([H2, 512], F32, name="he_ps", tag="tr", bufs=2)
            for k in range(KD):
                nc.tensor.matmul(
                    out=he_ps[:, : n1 - n0],
                    lhsT=wa_sb[k][:],
                    rhs=xT_sb[k][:, n0:n1],
                    start=(k == 0),
                    stop=(k == KD - 1),
                )
            nc.vector.tensor_copy(out=heT_sb[:, n0:n1], in_=he_ps[:, : n1 - n0])

        # ---------- he node-major per block; e_src -> ag_in[:, D:D+H]
        for b in range(NB):
            tr_ps = psum.tile([P, H2], F32, name="trh_ps", tag="ed", bufs=2)
            nc.tensor.transpose(
                out=tr_ps[:],
                in_=heT_sb[:, b * P : (b + 1) * P],
                identity=idh2[:],
            )
            nc.vector.tensor_copy(out=henm_sb[b][:], in_=tr_ps[:])
            nc.sync.dma_start(
                out=ag_in[b * P : (b + 1) * P, D : D + H], in_=henm_sb[b][:, 0:H]
            )

        # ---------- AllGather [h | e_src]
        nc.gpsimd.collective_compute(
            "AllGather",
            ALU.bypass,
            replica_groups=rg,
            ins=[ag_in.opt()],
            outs=[ag_out.opt()],
        )

        # ---------- edge phase
        for b in range(NB):
            mk_sb = sb.tile([P, CH, P], F32, name="mk_sb", tag="mk", bufs=2)
            mkT_sb = sb.tile([P, CH, P], F32, name="mkT_sb", tag="mkT", bufs=2)
            nc.sync.dma_start(out=mk_sb[:], in_=I["masks"][b])
            nc.sync.dma_start(out=mkT_sb[:], in_=I["maskTs"][b])
            out_ps = psum.tile([P, D], F32, name="out_ps", tag="big", bufs=2)
            den_ps = psum.tile([P, H], F32, name="den_ps", tag="den", bufs=2)
            for ch in range(CH):
                gt = sb.tile([P, ROW], F32, name="gt", tag="gt", bufs=4)
                nc.gpsimd.indirect_dma_start(
                    out=gt[:],
                    out_offset=None,
                    in_=ag_out[:],
                    in_offset=bass.IndirectOffsetOnAxis(
                        ap=gidx_sb[:, b, ch : ch + 1], axis=0
                    ),
                )
                ed_ps = psum.tile([P, H], F32, name="ed_ps", tag="ed", bufs=2)
                nc.tensor.matmul(
                    out=ed_ps[:],
                    lhsT=mkT_sb[:, ch, :],
                    rhs=henm_sb[b][:, H:H2],
                    start=True,
                    stop=True,
                )
                e_sb = sb.tile([P, H], F32, name="e_sb", tag="e_sb", bufs=4)
                nc.vector.tensor_add(out=e_sb[:], in0=gt[:, D : D + H], in1=ed_ps[:])
                e2_sb = sb.tile([P, H], F32, name="e2_sb", tag="e2_sb", bufs=4)
                nc.vector.scalar_tensor_tensor(
                    out=e2_sb[:],
                    in0=e_sb[:],
                    scalar=NEG,
                    in1=e_sb[:],
                    op0=ALU.mult,
                    op1=ALU.max,
                )
                ee_sb = sb.tile([P, H], F32, name="ee_sb", tag="ee_sb", bufs=4)
                nc.scalar.activation(ee_sb[:], e2_sb[:], ACT.Exp)
                gs = sb.tile([P, D], F32, name="gs", tag="gs", bufs=4)
                nc.vector.tensor_tensor(
                    out=gs[:].rearrange("p (h c) -> p h c", h=H),
                    in0=gt[:, 0:D].rearrange("p (h c) -> p h c", h=H),
                    in1=ee_sb[:].unsqueeze(2).to_broadcast([P, H, C]),
                    op=ALU.mult,
                )
                nc.tensor.matmul(
                    out=out_ps[:],
                    lhsT=mk_sb[:, ch, :],
                    rhs=gs[:],
                    start=(ch == 0),
                    stop=(ch == CH - 1),
                )
                nc.tensor.matmul(
                    out=den_ps[:],
                    lhsT=mk_sb[:, ch, :],
                    rhs=ee_sb[:],
                    start=(ch == 0),
                    stop=(ch == CH - 1),
                )

            # ----- block epilogue: normalize, bias, LN, relu
            den_sb = sb.tile([P, H], F32, name="den_sb", tag="den_sb", bufs=2)
            nc.vector.tensor_scalar_add(out=den_sb[:], in0=den_ps[:], scalar1=1e-16)
            rec_sb = sb.tile([P, H], F32, name="rec_sb", tag="rec_sb", bufs=2)
            nc.vector.reciprocal(out=rec_sb[:], in_=den_sb[:])
            y_sb = sb.tile([P, D], F32, name="y_sb", tag="y_sb", bufs=2)
            nc.vector.tensor_tensor(
                out=y_sb[:].rearrange("p (h c) -> p h c", h=H),
                in0=out_ps[:].rearrange("p (h c) -> p h c", h=H),
                in1=rec_sb[:].unsqueeze(2).to_broadcast([P, H, C]),
                op=ALU.mult,
            )
            if b_rep is not None:
                nc.vector.tensor_add(out=y_sb[:], in0=y_sb[:], in1=b_rep[:])
            # LN stats
            nmu = sb.tile([P, 1], F32, name="nmu", tag="nmu", bufs=2)
            nc.vector.tensor_reduce(
                out=nmu[:], in_=y_sb[:], axis=mybir.AxisListType.X,
                op=ALU.add, negate=True,
            )
            nc.scalar.mul(nmu[:], nmu[:], 1.0 / D)  # -mu
            sq_sb = sb.tile([P, D], F32, name="sq_sb", tag="sq_sb", bufs=2)
            ssq = sb.tile([P, 1], F32, name="ssq", tag="ssq", bufs=2)
            nc.scalar.activation(
                sq_sb[:], y_sb[:], ACT.Square,
                bias=nmu[:, 0:1], scale=1.0, accum_out=ssq[:, 0:1],
            )
            sd = sb.tile([P, 1], F32, name="sd", tag="sd", bufs=2)
            nc.scalar.activation(
                sd[:], ssq[:], ACT.Sqrt, bias=eps_sb[:, 0:1], scale=1.0 / D
            )
            rstd = sb.tile([P, 1], F32, name="rstd", tag="rstd", bufs=2)
            nc.vector.reciprocal(out=rstd[:], in_=sd[:])
            mm = sb.tile([P, 1], F32, name="mm", tag="mm", bufs=2)
            nc.vector.tensor_mul(out=mm[:], in0=nmu[:], in1=rstd[:])
            ln_sb = sb.tile([P, D], F32, name="ln_sb", tag="ln_sb", bufs=2)
            nc.scalar.activation(
                ln_sb[:], y_sb[:], ACT.Identity,
                bias=mm[:, 0:1], scale=rstd[:, 0:1],
            )
            if g_rep is not None:
                nc.vector.tensor_mul(out=ln_sb[:], in0=ln_sb[:], in1=g_rep[:])
            if be_rep is not None:
                nc.vector.tensor_add(out=ln_sb[:], in0=ln_sb[:], in1=be_rep[:])
            nc.scalar.activation(xn_sb[b][:], ln_sb[:], ACT.Relu)

        # ---------- transpose x_next into feature-major for next layer
        if l < L - 1:
            for b in range(NB):
                for k in range(KD):
                    t_ps = psum.tile([P, P], F32, name="t_ps", tag="tr", bufs=2)
                    nc.tensor.transpose(
                        out=t_ps[:],
                        in_=xn_sb[b][:, k * P : (k + 1) * P],
                        identity=id128[:],
                    )
                    nc.vector.tensor_copy(
                        out=xT_sb[k][:, b * P : (b + 1) * P], in_=t_ps[:]
                    )

    # ---------------- pooling (mean by graph) + FC
    ones_sb = res.tile([P, 1], F32, name="ones_sb")
    nc.vector.memset(ones_sb[:], 1.0)
    pm_pool = ctx.enter_context(tc.tile_pool(name="pm", bufs=2))
    pool_ps = psum.tile([G, D], F32, name="pool_ps", tag="big", bufs=2)
    cnt_ps = psum.tile([G, 1], F32, name="cnt_ps", tag="den", bufs=2)
    for b in range(NB):
        pm_sb = pm_pool.tile([P, G], F32, name="pm_sb", tag="pm_sb", bufs=2)
        nc.sync.dma_start(out=pm_sb[:], in_=I["poolmask"][b])
        nc.tensor.matmul(
            out=pool_ps[:], lhsT=pm_sb[:], rhs=xn_sb[b][:],
            start=(b == 0), stop=(b == NB - 1),
        )
        nc.tensor.matmul(
            out=cnt_ps[:], lhsT=pm_sb[:], rhs=ones_sb[:],
            start=(b == 0), stop=(b == NB - 1),
        )
    pool_sb = res.tile([G, D + 1], F32, name="pool_sb")
    nc.vector.tensor_copy(out=pool_sb[:, 0:D], in_=pool_ps[:])
    nc.vector.tensor_copy(out=pool_sb[:, D : D + 1], in_=cnt_ps[:])
    nc.sync.dma_start(out=ar_in[:], in_=pool_sb[:])
    nc.gpsimd.collective_compute(
        "AllReduce",
        ALU.add,
        replica_groups=rg,
        ins=[ar_in.opt()],
        outs=[ar_out.opt()],
    )
    pf_sb = res.tile([G, D + 1], F32, name="pf_sb")
    nc.sync.dma_start(out=pf_sb[:], in_=ar_out[:])
    cntf = res.tile([G, 1], F32, name="cntf")
    nc.vector.tensor_scalar_max(out=cntf[:], in0=pf_sb[:, D : D + 1], scalar1=1.0)
    crec = res.tile([G, 1], F32, name="crec")
    nc.vector.reciprocal(out=crec[:], in_=cntf[:])
    pn_sb = res.tile([G, D], F32, name="pn_sb")
    nc.vector.tensor_tensor(
        out=pn_sb[:], in0=pf_sb[:, 0:D],
        in1=crec[:].to_broadcast([G, D]), op=ALU.mult,
    )
    # transpose pooled -> (KD chunks of (128, G))
    pT_sb = res.tile([P, KD, G], F32, name="pT_sb")
    for k in range(KD):
        t2_ps = psum.tile([P, G], F32, name="t2_ps", tag="tr", bufs=2)
        nc.tensor.transpose(
            out=t2_ps[:], in_=pn_sb[:, k * P : (k + 1) * P], identity=idG[:]
        )
        nc.vector.tensor_copy(out=pT_sb[:, k, :], in_=t2_ps[:])
    fcw_sb = res.tile([P, KD, OUT], F32, name="fcw_sb")
    nc.sync.dma_start(
        out=fcw_sb[:], in_=I["fc_W"][:].rearrange("k p o -> p k o")
    )
    fc_ps = psum.tile([G, OUT], F32, name="fc_ps", tag="big", bufs=2)
    for k in range(KD):
        nc.tensor.matmul(
            out=fc_ps[:], lhsT=pT_sb[:, k, :], rhs=fcw_sb[:, k, :],
            start=(k == 0), stop=(k == KD - 1),
        )
    o_sb = res.tile([G, OUT], F32, name="o_sb")
    if not meta["skip_fcb"]:
        fcb_rep = res.tile([P, OUT], F32, name="fcb_rep")
        nc.sync.dma_start(out=fcb_rep[:], in_=I["fcb_rep"][:])
        nc.vector.tensor_add(out=o_sb[:], in0=fc_ps[:], in1=fcb_rep[0:G, :])
    else:
        nc.vector.tensor_copy(out=o_sb[:], in_=fc_ps[:])
    nc.sync.dma_start(out=out_ap[:], in_=o_sb[:])
    ctx.close()


# --------------------------------------------------------------------------
# Entry point
# --------------------------------------------------------------------------


def kernel(**inputs):
    global LAST_RESULTS
    cfg = _full_cfg()
    in_maps, meta = _prep(inputs, cfg)

    nc = bacc.Bacc(
        "TRN2",
        target_bir_lowering=False,
        debug=False,
        enable_asserts=False,
        num_devices=cfg["n_cores"],
    )
    I = {}
    for name, arr in in_maps[0].items():
        I[name] = nc.dram_tensor(
            name, arr.shape, mybir.dt.from_np(arr.dtype), kind="ExternalInput"
        ).ap()
    out_ap = nc.dram_tensor(
        "out", (cfg["G"], cfg["OUT"]), F32, kind="ExternalOutput"
    ).ap()

    with tile.TileContext(nc) as tc:
        build(tc, cfg, meta, I, out_ap)
    nc.compile()

    trace = bool(int(os.environ.get("GAT_TRACE", "0")))
    res = run_bass_kernel_spmd(
        nc,
        in_maps,
        core_ids=list(range(cfg["n_cores"])),
        trace=trace,
    )
    LAST_RESULTS = res
    return np.asarray(res.results[0]["out"])

